# revision 1
# baseline (speedup 1.0000x reference)
"""DropBlock kernel for Trainium2, 8 NeuronCores, batch-sharded data parallel.

Reference computation (B,C,H,W = 128,64,56,56, block=5, gamma=0.02):
    mask    = (noise < gamma)                       # (B,C,52,52) corner drops
    dilated = maxpool5x5_full_pad(mask)             # (B,C,56,56)
    block_mask = 1 - dilated
    out = block_mask * x * (numel / sum(block_mask))

Kernel formulation (exact, no mask materialization in f32):
    d = noise - gamma_lo  (f32 subtract; sign/zero exact by Sterbenz, then
                           bf16 cast which preserves sign, never rounds to 0)
    block_mask[h,w] = ( min_{5x5 window}(d) > 0 )   # min-pool == dilated drop
    count = sum(block_mask) via fused accum, AllReduce across 8 cores.

Each core: 16 batches x 64 ch = 1024 images -> 8 tiles of 128 images
(images on partitions, image pixels along the free dimension).  The 5-wide
separable min uses log-step shifts (3 tensor_tensor ops per axis) on
1.0-padded buffers so no boundary special cases are needed.
"""

import sys

sys.path.insert(0, "/opt/trn_rl_repo")

import numpy as np

import concourse.bacc as bacc
import concourse.bass as bass
import concourse.tile as tile
import concourse.mybir as mybir
from concourse import bass_isa
from concourse.bass_utils import run_bass_kernel_spmd

N_CORES = 8
B, C, H, W = 128, 64, 56, 56
BLK = 5
GAMMA = 0.02
NH, NW = H - (BLK - 1), W - (BLK - 1)  # 52, 52 noise dims
B_SH = B // N_CORES  # 16 batches per core
IMGS = B_SH * C  # 1024 images per core
P = 128  # partitions
NTILES = IMGS // P  # 8 tiles per core
NPIX = NH * NW  # 2704 noise pixels/image
OPIX = H * W  # 3136 out pixels/image
TROWS = NH + 2 * (BLK - 1)  # 60 rows in padded vertical buffer
TFLAT = TROWS * NW  # 3120
VPW = NW + 2 * (BLK - 1)  # 60 cols in padded horizontal buffer (4+52+4)
COUNT_M = float(B * C * H * W)  # 25690112.0

# Largest f32 strictly below 0.02f: keep <=> noise >= 0.02f <=> noise-g' > 0,
# so the mask is Relu(Sign(min-pool(noise-g'))) with exact {0,1} handling.
GAMMA_LO = float(np.nextafter(np.float32(GAMMA), np.float32(0)))

F32 = mybir.dt.float32
BF16 = mybir.dt.bfloat16
MIN = mybir.AluOpType.min
MULT = mybir.AluOpType.mult
FP8 = mybir.dt.float8e4

X_PREFETCH = 8

_CACHE = {}


def _build(single_core=False, repeat=1, no_cc=False):
    """Build + compile the SPMD bass module once.

    single_core=True builds a collective-free variant (the per-core count is
    used directly as the global count) for cost-model simulation only.
    repeat>1 unrolls the whole pipeline k times (benchmarking only).
    no_cc=True skips the AllReduce on the 8-core build (timing probe only —
    results are wrong by the per-core/global count ratio).
    """
    nc = bacc.Bacc("TRN2", target_bir_lowering=False, debug=False,
                   num_devices=1 if single_core else N_CORES)
    noise_ap = nc.dram_tensor("noise", [IMGS, NPIX], F32,
                              kind="ExternalInput").ap()
    x_ap = nc.dram_tensor("x", [IMGS, OPIX], F32, kind="ExternalInput").ap()
    out_ap = nc.dram_tensor("out", [IMGS, OPIX], F32,
                            kind="ExternalOutput").ap()

    with tile.TileContext(nc) as tc:
        with (
            tc.tile_pool(name="nraw", bufs=2) as nraw_pool,
            tc.tile_pool(name="work", bufs=5) as work_pool,
            tc.tile_pool(name="vp", bufs=2) as vp_pool,
            tc.tile_pool(name="dmin", bufs=2) as dmin_pool,
            tc.tile_pool(name="mask", bufs=1) as mask_pool,
            tc.tile_pool(name="stats", bufs=1) as stats_pool,
            tc.tile_pool(name="xio", bufs=X_PREFETCH) as x_pool,
            tc.tile_pool(name="dram", bufs=1, space="DRAM") as dram_pool,
        ):
            mask_store = mask_pool.tile([P, NTILES * OPIX], FP8)
            partials = stats_pool.tile([P, NTILES], F32)
            gbias = stats_pool.tile([P, 1], F32)
            nc.vector.memset(gbias[:], -GAMMA_LO)
            # warm the ACT function tables on a 1-element tile during the
            # DMA lead-in; otherwise LoadActFuncSet (~1.3us) lands in front
            # of the first real subtract and stalls the DVE pipeline.
            warm = stats_pool.tile([P, 1], F32)
            nc.scalar.activation(warm[:], gbias[:],
                                 mybir.ActivationFunctionType.Identity,
                                 bias=gbias[:, 0:1])
            nc.scalar.activation(warm[:], warm[:],
                                 mybir.ActivationFunctionType.Sign)
            nc.scalar.activation(warm[:], warm[:],
                                 mybir.ActivationFunctionType.Relu)

            for rep in range(repeat):
                _emit_once(nc, tc, noise_ap, x_ap, out_ap, mask_store,
                           partials, gbias, nraw_pool, work_pool,
                           vp_pool, dmin_pool, stats_pool, x_pool,
                           dram_pool, single_core or no_cc, rep)

    nc.compile()
    return nc


def _emit_once(nc, tc, noise_ap, x_ap, out_ap, mask_store, partials, gbias,
               nraw_pool, work_pool, vp_pool, dmin_pool, stats_pool,
               x_pool, dram_pool, single_core, rep):
    # ---------------- phase 1: block mask + counts ----------------
    xts = {}
    for t in range(NTILES):
        nraw = nraw_pool.tile([P, NPIX], F32, name=f"nraw{rep}_{t}",
                              tag="nraw")
        if t == 0:
            # halve the cold-start DMA latency: the first subtract/min half
            # can begin as soon as rows 0..25 have landed
            nc.sync.dma_start(nraw[:, 0:NPIX // 2],
                              noise_ap[bass.ts(t, P), 0:NPIX // 2])
            nc.sync.dma_start(nraw[:, NPIX // 2:NPIX],
                              noise_ap[bass.ts(t, P), NPIX // 2:NPIX])
        else:
            nc.sync.dma_start(nraw[:], noise_ap[bass.ts(t, P), :])
        if t < X_PREFETCH:
            # prefetch x tiles early on the same queue, behind this tile's
            # noise load; the sync queue stays ahead of the ~10us/tile
            # compute cadence and the scalar queue stays compute-only.
            xts[t] = x_pool.tile([P, OPIX], F32, name=f"xt{rep}_{t}",
                                 tag="xt")
            nc.sync.dma_start(xts[t][:], x_ap[bass.ts(t, P), :])

        # T: (60,52) bf16, rows 0..3 / 56..59 = 1.0 pad,
        # rows 4..55 = noise - gamma_lo
        tb = work_pool.tile([P, TFLAT], BF16, name=f"tb{rep}_{t}", tag="w")
        nc.gpsimd.memset(tb[:, 0:(BLK - 1) * NW], 1.0)
        nc.gpsimd.memset(tb[:, (NH + BLK - 1) * NW:TFLAT], 1.0)
        if t == 0:
            # tile 0's subtract runs on DVE itself (TS 2x mode, f32->bf16),
            # in two halves chasing the two DMA halves.  T rows 4..29 come
            # from noise rows 0..25, rows 30..55 from rows 26..51.
            nc.vector.tensor_scalar(
                tb[:, (BLK - 1) * NW:30 * NW], nraw[:, 0:NPIX // 2],
                -GAMMA_LO, None, mybir.AluOpType.add)
            nc.vector.tensor_scalar(
                tb[:, 30 * NW:(NH + BLK - 1) * NW], nraw[:, NPIX // 2:NPIX],
                -GAMMA_LO, None, mybir.AluOpType.add)
        else:
            nc.scalar.activation(
                tb[:, (BLK - 1) * NW:(NH + BLK - 1) * NW], nraw[:],
                mybir.ActivationFunctionType.Identity, bias=gbias[:, 0:1])

        # vertical min pool, log-step: windows of 2, 4, then 5.
        # A rows 0..2 and 56..58 are mins of pad rows only (= 1.0): memset
        # them and run DVE only over the data-dependent rows 3..55.
        a = work_pool.tile([P, (TROWS - 1) * NW], BF16, name=f"a{rep}_{t}",
                        tag="w")  # 59 rows
        nc.gpsimd.memset(a[:, 0:3 * NW], 1.0)
        nc.gpsimd.memset(a[:, 56 * NW:(TROWS - 1) * NW], 1.0)
        if t == 0:
            # A rows 3..28 need only T rows 3..29 (first subtract half)
            nc.vector.tensor_tensor(
                a[:, 3 * NW:29 * NW], tb[:, 3 * NW:29 * NW],
                tb[:, 4 * NW:30 * NW], MIN)
            nc.vector.tensor_tensor(
                a[:, 29 * NW:56 * NW], tb[:, 29 * NW:56 * NW],
                tb[:, 30 * NW:57 * NW], MIN)
        else:
            nc.vector.tensor_tensor(
                a[:, 3 * NW:56 * NW], tb[:, 3 * NW:56 * NW],
                tb[:, 4 * NW:57 * NW], MIN)
        # B row 0 = min(A0, A2) = 1.0 likewise.
        bt = work_pool.tile([P, (TROWS - 3) * NW], BF16, name=f"bt{rep}_{t}",
                         tag="w")  # 57 rows
        nc.gpsimd.memset(bt[:, 0:NW], 1.0)
        nc.vector.tensor_tensor(
            bt[:, NW:(TROWS - 3) * NW], a[:, NW:(TROWS - 3) * NW],
            a[:, 3 * NW:(TROWS - 1) * NW], MIN)
        # V[r] = min(B[r], T[r+4]), r in 0..55 -> into padded Vp
        vp = vp_pool.tile([P, H * VPW], BF16, name=f"vp{rep}_{t}", tag="vp")
        vp3 = vp[:].rearrange("p (h w) -> p h w", w=VPW)
        nc.gpsimd.memset(vp3[:, :, 0:BLK - 1], 1.0)
        nc.gpsimd.memset(vp3[:, :, W:VPW], 1.0)
        bt3 = bt[:].rearrange("p (h w) -> p h w", w=NW)
        tb3 = tb[:].rearrange("p (h w) -> p h w", w=NW)
        nc.vector.tensor_tensor(
            vp3[:, :, BLK - 1:BLK - 1 + NW], bt3[:, 0:H, :],
            tb3[:, BLK - 1:TROWS, :], MIN)

        # horizontal min pool, log-step (flat shifted APs; the out-of-row
        # tail elements are junk but never read)
        HV = H * VPW
        a2 = work_pool.tile([P, HV], BF16, name=f"a2{rep}_{t}", tag="w")
        nc.vector.tensor_tensor(
            a2[:, 0:HV - 1], vp[:, 0:HV - 1], vp[:, 1:HV], MIN)
        b2 = work_pool.tile([P, HV], BF16, name=f"b2{rep}_{t}", tag="w")
        nc.vector.tensor_tensor(
            b2[:, 0:HV - 2], a2[:, 0:HV - 2], a2[:, 2:HV], MIN)
        b23 = b2[:].rearrange("p (h w) -> p h w", w=VPW)
        dm = dmin_pool.tile([P, OPIX], BF16, name=f"dm{rep}_{t}", tag="dm")
        dm3 = dm[:].rearrange("p (h w) -> p h w", w=W)
        nc.vector.tensor_tensor(
            dm3[:, :, :], b23[:, :, 0:W], vp3[:, :, BLK - 1:VPW], MIN)

        # block_mask = (dmin > 0); count per partition into partials[:, t].
        # Tiles 0..n-2: ACT Relu(Sign(d)) with fused f32 accum, overlapped
        # with later tiles' DVE work.  Last tile: DVE is_gt + f32 reduce so
        # the count (-> collective -> scale) skips the ACT queue.  (The DVE
        # tensor_scalar accum_out accumulates in the fp8 OUT dtype and
        # saturates, so the reduce is a separate exact op.)
        mslice = mask_store[:, t * OPIX:(t + 1) * OPIX]
        if t == NTILES - 1:
            nc.vector.tensor_scalar(mslice, dm[:], 0.0, 1.0,
                                    mybir.AluOpType.is_gt, MULT)
            nc.vector.tensor_reduce(partials[:, t:t + 1], mslice,
                                    mybir.AxisListType.X,
                                    mybir.AluOpType.add)
        else:
            nc.scalar.activation(dm[:], dm[:],
                                 mybir.ActivationFunctionType.Sign)
            nc.scalar.activation(
                mslice, dm[:], mybir.ActivationFunctionType.Relu,
                accum_out=partials[:, t:t + 1])

    # ------------- global count -> scale = M / count_ones -------------
    # partials[0:7] are reduced while tile 7 is still computing; only the
    # tiny add of tile 7's count sits on the critical chain.
    phead = stats_pool.tile([P, 1], F32, name=f"phead{rep}", tag="phead")
    nc.vector.tensor_reduce(phead[:], partials[:, 0:NTILES - 1],
                            mybir.AxisListType.X, mybir.AluOpType.add)
    ptot = stats_pool.tile([P, 1], F32, name=f"ptot{rep}", tag="ptot")
    nc.vector.tensor_tensor(ptot[:], phead[:],
                            partials[:, NTILES - 1:NTILES],
                            mybir.AluOpType.add)
    pall = stats_pool.tile([P, 1], F32, name=f"pall{rep}", tag="pall")
    nc.gpsimd.partition_all_reduce(pall[:], ptot[:], channels=P,
                                   reduce_op=bass_isa.ReduceOp.add)
    if single_core:
        tot_sb = pall
    else:
        cc_in = dram_pool.tile([P, 1], F32, name=f"cc_in{rep}", tag="cc_in")
        cc_out = dram_pool.tile([P, 1], F32, name=f"cc_out{rep}",
                                tag="cc_out")
        nc.sync.dma_start(cc_in[:], pall[:])
        nc.gpsimd.collective_compute(
            "AllReduce", mybir.AluOpType.add,
            replica_groups=[list(range(N_CORES))],
            ins=[cc_in.opt()], outs=[cc_out.opt()])
        tot_sb = stats_pool.tile([P, 1], F32, name=f"tot{rep}", tag="tot")
        nc.sync.dma_start(tot_sb[:], cc_out[:])
    recip = stats_pool.tile([P, 1], F32, name=f"recip{rep}", tag="recip")
    nc.vector.reciprocal(recip[:], tot_sb[:])
    scale_sb = stats_pool.tile([P, 1], F32, name=f"scale{rep}", tag="scale")
    nc.vector.tensor_scalar_mul(scale_sb[:], recip[:], COUNT_M)

    # ---------------- phase 2: out = (x*scale)*mask ----------------
    for t in range(NTILES):
        if t in xts:
            xt = xts[t]
        else:
            xt = x_pool.tile([P, OPIX], F32, name=f"xt{rep}_{t}", tag="xt")
            nc.scalar.dma_start(xt[:], x_ap[bass.ts(t, P), :])
        nc.vector.scalar_tensor_tensor(
            xt[:], xt[:], scale_sb[:, 0:1],
            mask_store[:, t * OPIX:(t + 1) * OPIX], MULT, MULT)
        # alternate stores across both HWDGE queues (scalar queue is idle
        # in phase 2) so the final drain is paced by aggregate DMA BW, not
        # one queue's serialization
        eng = nc.sync if t % 2 == 0 else nc.scalar
        eng.dma_start(out_ap[bass.ts(t, P), :], xt[:])


def _get_nc():
    if "nc" not in _CACHE:
        _CACHE["nc"] = _build()
    return _CACHE["nc"]


def kernel(x: np.ndarray, noise: np.ndarray) -> np.ndarray:
    x = np.asarray(x, dtype=np.float32)
    noise = np.asarray(noise, dtype=np.float32)
    assert x.shape == (B, C, H, W) and noise.shape == (B, C, NH, NW)
    nc = _get_nc()
    in_maps = []
    for i in range(N_CORES):
        xs = np.ascontiguousarray(x[i * B_SH:(i + 1) * B_SH]).reshape(
            IMGS, OPIX)
        ns = np.ascontiguousarray(noise[i * B_SH:(i + 1) * B_SH]).reshape(
            IMGS, NPIX)
        in_maps.append({"x": xs, "noise": ns})
    res = run_bass_kernel_spmd(nc, in_maps, list(range(N_CORES)))
    out = np.empty((B, C, H, W), dtype=np.float32)
    for i in range(N_CORES):
        out[i * B_SH:(i + 1) * B_SH] = res.results[i]["out"].reshape(
            B_SH, C, H, W)
    return out



# revision 18
# speedup vs baseline: 1.0211x; 1.0211x over previous
"""DropBlock kernel for Trainium2, 8 NeuronCores, batch-sharded data parallel.

Reference computation (B,C,H,W = 128,64,56,56, block=5, gamma=0.02):
    mask    = (noise < gamma)                       # (B,C,52,52) corner drops
    dilated = maxpool5x5_full_pad(mask)             # (B,C,56,56)
    block_mask = 1 - dilated
    out = block_mask * x * (numel / sum(block_mask))

Kernel formulation (exact):
    s = Sign(noise - gamma_lo) in {-1, 0, +1}  (ACT, f32 internal, bf16 out;
        gamma_lo = largest f32 < 0.02f so keep <=> noise >= 0.02f <=> s > 0)
    block_mask[h,w] = ( min_{5x5 window}(s) > 0 )   # min-pool == dilated drop
    mask = Relu(minpool(s)) in {0,1}; count = fused ACT accum; AllReduce.

Engine split (from TimelineSim occupancy analysis — DVE was the 81%-busy
bottleneck, Pool 10%, ACT 42%, PE 0%):
    ACT : sign, relu+count-accum
    DVE : vertical min (a, bt, vp), horizontal b2, dm rows 25..55
    Pool: horizontal a2, dm rows 0..24, AllReduce trigger
    PE  : ones[128,128] @ ptot -> per-core count broadcast into PSUM
Pad-carrying buffers (tb/a/bt, vp) are role-stable across tiles (bufs=1/2
tags) so the +1.0 pads are memset ONCE — per-tile pad memsets on the Pool
queue were serializing the whole pipeline (11.8us/tile -> ~7.8us/tile).
Phase 2 multiplies alternate DVE/Pool; stores alternate the two HWDGE
queues.  Tile 7's tail (b2/dm/relu) is split in row-halves on DVE to
shorten the critical path into the collective.
"""

import sys

sys.path.insert(0, "/opt/trn_rl_repo")

import numpy as np

import concourse.bacc as bacc
import concourse.bass as bass
import concourse.tile as tile
import concourse.mybir as mybir
from concourse.bass_utils import run_bass_kernel_spmd

N_CORES = 8
B, C, H, W = 128, 64, 56, 56
BLK = 5
GAMMA = 0.02
NH, NW = H - (BLK - 1), W - (BLK - 1)  # 52, 52 noise dims
B_SH = B // N_CORES  # 16 batches per core
IMGS = B_SH * C  # 1024 images per core
P = 128  # partitions
NTILES = IMGS // P  # 8 tiles per core
NPIX = NH * NW  # 2704 noise pixels/image
OPIX = H * W  # 3136 out pixels/image
TROWS = NH + 2 * (BLK - 1)  # 60 rows in padded vertical buffer
TFLAT = TROWS * NW  # 3120
VPW = NW + 2 * (BLK - 1)  # 60 cols in padded horizontal buffer (4+52+4)
HV = H * VPW  # 3360
COUNT_M = float(B * C * H * W)  # 25690112.0

# Largest f32 strictly below 0.02f: keep <=> noise >= 0.02f <=> noise-g' > 0.
GAMMA_LO = float(np.nextafter(np.float32(GAMMA), np.float32(0)))

F32 = mybir.dt.float32
BF16 = mybir.dt.bfloat16
FP8 = mybir.dt.float8e4
MIN = mybir.AluOpType.min
MULT = mybir.AluOpType.mult
ADD = mybir.AluOpType.add
SIGN = mybir.ActivationFunctionType.Sign
RELU = mybir.ActivationFunctionType.Relu

DM_POOL_ROWS = 20  # dm rows on Pool; safe with vp bufs=3 (WAR 3 tiles back)

_CACHE = {}


def _build(single_core=False, repeat=1, no_cc=False):
    """Build + compile the SPMD bass module once.

    single_core=True builds a collective-free variant (per-core count used
    as the global count) for cost-model simulation only.  repeat>1 unrolls
    the pipeline k times (benchmarking only).  no_cc=True skips the
    AllReduce on the 8-core build (timing probe only).
    """
    nc = bacc.Bacc("TRN2", target_bir_lowering=False, debug=False,
                   num_devices=1 if single_core else N_CORES)
    noise_ap = nc.dram_tensor("noise", [IMGS, NPIX], F32,
                              kind="ExternalInput").ap()
    x_ap = nc.dram_tensor("x", [IMGS, OPIX], F32, kind="ExternalInput").ap()
    out_ap = nc.dram_tensor("out", [IMGS, OPIX], F32,
                            kind="ExternalOutput").ap()

    with tile.TileContext(nc) as tc:
        with (
            tc.tile_pool(name="nraw", bufs=3) as nraw_pool,
            tc.tile_pool(name="work", bufs=1) as work_pool,
            tc.tile_pool(name="vp", bufs=2) as vp_pool,
            tc.tile_pool(name="dmin", bufs=2) as dmin_pool,
            tc.tile_pool(name="mask", bufs=1) as mask_pool,
            tc.tile_pool(name="stats", bufs=1) as stats_pool,
            tc.tile_pool(name="xio", bufs=8) as x_pool,
            tc.tile_pool(name="dram", bufs=1, space="DRAM") as dram_pool,
        ):
            mask_store = mask_pool.tile([P, NTILES * OPIX], FP8)
            # 16 partial-count columns: per-tile dve-rows (0..6) and
            # pool-rows (7..13) relu parts + tile-7 halves (14, 15)
            partials = stats_pool.tile([P, 16], F32)
            gbias = stats_pool.tile([P, 1], F32)
            nc.vector.memset(gbias[:], -GAMMA_LO)
            # warm the ACT function tables during the DMA lead-in so
            # LoadActFuncSet (~1.3us) doesn't stall the first real sign.
            warm = stats_pool.tile([P, 1], F32)
            warm2 = stats_pool.tile([P, 1], F32)
            nc.scalar.activation(warm[:], gbias[:], SIGN, bias=gbias[:, 0:1])
            nc.scalar.activation(warm[:], warm[:], RELU,
                                 accum_out=warm2[:, 0:1])

            pools = dict(nraw=nraw_pool, work=work_pool, vp=vp_pool,
                         dmin=dmin_pool, x=x_pool,
                         stats=stats_pool, dram=dram_pool)
            for rep in range(repeat):
                _emit_once(nc, tc, noise_ap, x_ap, out_ap, mask_store,
                           partials, gbias, pools,
                           single_core or no_cc, rep)

    nc.compile()
    return nc


def _emit_once(nc, tc, noise_ap, x_ap, out_ap, mask_store, partials, gbias,
               pools, single_core, rep):
    nraw_pool = pools["nraw"]
    work_pool = pools["work"]
    vp_pool = pools["vp"]
    dmin_pool = pools["dmin"]
    x_pool = pools["x"]
    stats_pool = pools["stats"]
    dram_pool = pools["dram"]

    # ---------------- phase 1: block mask + counts ----------------
    xts = {}
    state = {}  # per-tile handles for the staggered emission
    pre = {}
    if True:
        # pre-allocate the pad-carrying buffers and memset their +1.0 pads
        # ONCE, up-front, while Pool is otherwise idle -- in-loop memsets
        # land behind a2 ops on the in-order Pool queue and stall DVE
        pre["tb"] = [work_pool.tile([P, TFLAT], BF16, name=f"tbp{rep}_{i}",
                                    tag="tb", bufs=2) for i in range(2)]
        pre["a"] = [work_pool.tile([P, 59 * NW], BF16, name=f"ap{rep}_0",
                                   tag="a", bufs=1)]
        pre["vp"] = [vp_pool.tile([P, HV], BF16, name=f"vpp{rep}_{i}",
                                  tag="vp") for i in range(2)]
        if rep == 0:
            for tbp in pre["tb"]:
                nc.gpsimd.memset(tbp[:, 0:(BLK - 1) * NW], 1.0)
                nc.gpsimd.memset(tbp[:, (NH + BLK - 1) * NW:TFLAT], 1.0)
            ap = pre["a"][0]
            nc.gpsimd.memset(ap[:, 0:3 * NW], 1.0)
            nc.gpsimd.memset(ap[:, 56 * NW:59 * NW], 1.0)
            for vpp in pre["vp"]:
                vp3 = vpp[:].rearrange("p (h w) -> p h w", w=VPW)
                nc.gpsimd.memset(vp3[:, :, 0:BLK - 1], 1.0)
                nc.gpsimd.memset(vp3[:, :, W:VPW], 1.0)

    def emit_sign(t):
        # noise in two half-DMAs so the sign (and tile 0's whole chain)
        # can chase the load; halves also shrink the nraw ring to 3x half
        nh0 = nraw_pool.tile([P, NPIX // 2], F32, name=f"nh{rep}_{t}a",
                             tag="nh")
        nh1 = nraw_pool.tile([P, NPIX // 2], F32, name=f"nh{rep}_{t}b",
                             tag="nh")
        nc.sync.dma_start(nh0[:], noise_ap[bass.ts(t, P), 0:NPIX // 2])
        nc.sync.dma_start(nh1[:], noise_ap[bass.ts(t, P), NPIX // 2:NPIX])
        # x loads ride the OTHER HWDGE queue (ACT's) so noise loads are
        # never queued behind them: noise then paces with the compute
        # pipeline and x fills the leftover DMA bandwidth
        xts[t] = x_pool.tile([P, OPIX], F32, name=f"xt{rep}_{t}", tag="xt")
        with tc.tile_wait_until(t * 10.0 + 5.0):
            nc.scalar.dma_start(xts[t][:, 0:OPIX // 2],
                                x_ap[bass.ts(t, P), 0:OPIX // 2])
            nc.scalar.dma_start(xts[t][:, OPIX // 2:OPIX],
                                x_ap[bass.ts(t, P), OPIX // 2:OPIX])

        # T: (60,52) bf16, rows 0..3 / 56..59 = +1.0 pad (memset once),
        # rows 4..55 = sign(noise - gamma_lo) in {-1, 0, +1}
        tb = pre["tb"][t] if t < 2 else work_pool.tile(
            [P, TFLAT], BF16, name=f"tb{rep}_{t}", tag="tb", bufs=2)
        nc.scalar.activation(tb[:, (BLK - 1) * NW:30 * NW], nh0[:], SIGN,
                             bias=gbias[:, 0:1])
        nc.scalar.activation(tb[:, 30 * NW:(NH + BLK - 1) * NW], nh1[:],
                             SIGN, bias=gbias[:, 0:1])
        return tb

    def emit_vert(t, tb):
        # vertical min pool, log-step: a = min2; bt[j] = min(a[j+1], a[j+3])
        # covers rows j+1..j+4; V[r] = min(a[r], bt[r]) covers r..r+4.
        # Note tb is dead after `a` (V reads a/bt only), so the sign of
        # tile t+1 only waits on a(t-1) -- keeps noise loads 2 tiles ahead.
        a = pre["a"][0] if t == 0 else work_pool.tile(
            [P, 59 * NW], BF16, name=f"a{rep}_{t}", tag="a", bufs=1)
        if t == 0:
            # chase the two sign halves: rows 3..28 need only T rows 3..29
            nc.vector.tensor_tensor(a[:, 3 * NW:29 * NW],
                                    tb[:, 3 * NW:29 * NW],
                                    tb[:, 4 * NW:30 * NW], MIN)
            nc.vector.tensor_tensor(a[:, 29 * NW:56 * NW],
                                    tb[:, 29 * NW:56 * NW],
                                    tb[:, 30 * NW:57 * NW], MIN)
        else:
            nc.vector.tensor_tensor(a[:, 3 * NW:56 * NW],
                                    tb[:, 3 * NW:56 * NW],
                                    tb[:, 4 * NW:57 * NW], MIN)
        bt = work_pool.tile([P, H * NW], BF16, name=f"bt{rep}_{t}",
                            tag="bt", bufs=1)  # rows j=0..55 == min4[j+1]
        nc.vector.tensor_tensor(bt[:, 0:H * NW], a[:, NW:57 * NW],
                                a[:, 3 * NW:59 * NW], MIN)
        # V[r] = min(a[r], bt[r]), r in 0..55 -> into padded Vp
        vp = pre["vp"][t] if t < 2 else vp_pool.tile(
            [P, HV], BF16, name=f"vp{rep}_{t}", tag="vp")
        vp3 = vp[:].rearrange("p (h w) -> p h w", w=VPW)
        a3 = a[:].rearrange("p (h w) -> p h w", w=NW)
        bt3 = bt[:].rearrange("p (h w) -> p h w", w=NW)
        nc.vector.tensor_tensor(vp3[:, :, BLK - 1:BLK - 1 + NW],
                                a3[:, 0:H, :], bt3[:, 0:H, :], MIN)
        return vp

    def emit_a2(t, vp):
        # horizontal min step 1 on Pool (flat shifted APs; junk tails
        # are never read)
        a2 = work_pool.tile([P, HV], BF16, name=f"a2{rep}_{t}", tag="a2",
                            bufs=2)
        nc.vector.tensor_tensor(a2[:, 0:HV - 1], vp[:, 0:HV - 1],
                                vp[:, 1:HV], MIN)
        return a2

    B2_POOL = 0  # b2 on DVE: Pool results consumed same-period stall the DVE chain

    def emit_tail(t, vp, a2, rows=None, b2_pool=B2_POOL):
        """b2 (flat split Pool/DVE), dm (DVE) for row range [r0, r1)."""
        r0, r1 = (0, H) if rows is None else rows
        b2 = state[t].get("b2")
        if b2 is None:
            b2 = work_pool.tile([P, HV], BF16, name=f"b2{rep}_{t}",
                                tag="b2", bufs=1)
            state[t]["b2"] = b2
            dm = dmin_pool.tile([P, OPIX], BF16, name=f"dm{rep}_{t}",
                                tag="dm")
            state[t]["dm"] = dm
        dm = state[t]["dm"]
        # b2[i] = min(a2[i], a2[i+2]) -- any flat split is valid; dm rows
        # r0..r1 read b2[r0*VPW : (r1-1)*VPW + W + 1]
        lo, hi = r0 * VPW, min((r1 - 1) * VPW + W + 2, HV - 2)
        nc.vector.tensor_tensor(b2[:, lo:hi], a2[:, lo:hi],
                                a2[:, lo + 2:hi + 2], MIN)
        b23 = b2[:].rearrange("p (h w) -> p h w", w=VPW)
        vp3 = vp[:].rearrange("p (h w) -> p h w", w=VPW)
        dm3 = dm[:].rearrange("p (h w) -> p h w", w=W)
        nc.vector.tensor_tensor(dm3[:, r0:r1, :], b23[:, r0:r1, 0:W],
                                vp3[:, r0:r1, BLK - 1:VPW], MIN)
        return dm

    def emit_relu(t, col, rows=None):
        """mask = Relu(dm) into mask_store, count accum into partials."""
        r0, r1 = (0, H) if rows is None else rows
        dm = state[t]["dm"]
        msl = mask_store[:].rearrange("p (t h w) -> p t h w", t=NTILES, w=W)
        dm3 = dm[:].rearrange("p (h w) -> p h w", w=W)
        nc.scalar.activation(msl[:, t, r0:r1, :], dm3[:, r0:r1, :], RELU,
                             accum_out=partials[:, col:col + 1])

    # staggered emission: iteration t emits sign(t), vert(t), a2(t),
    # tail(t-1), relu(t-2)
    RB = 10.0  # pseudo-ms per tile for scheduler ordering only
    for t in range(NTILES):
        state[t] = {}
        with tc.tile_wait_until(t * RB):
            tb = emit_sign(t)
            vp = emit_vert(t, tb)
            state[t]["vp"] = vp
            if t < NTILES - 1:
                state[t]["a2"] = emit_a2(t, vp)
        if 0 <= t - 1 < NTILES - 1:
            with tc.tile_wait_until(t * RB + 0.25 * RB):
                emit_tail(t - 1, state[t - 1]["vp"], state[t - 1]["a2"])
        if t >= 2:
            with tc.tile_wait_until(t * RB + 0.3 * RB):
                emit_relu(t - 2, t - 2)

    # finish the staggered relus, then run tile 7's horizontal chain in
    # row-thirds entirely on DVE (finer chase -> shorter critical path
    # into the count)
    vp7 = state[7]["vp"]
    R3 = [(0, 19), (19, 38), (38, H)]
    with tc.tile_wait_until(8 * RB):
        emit_relu(5, 5)
        emit_relu(6, 6)
        a27 = work_pool.tile([P, HV], BF16, name=f"a27{rep}", tag="a2",
                             bufs=2)
        state[7]["a2"] = a27
        for i, (r0, r1) in enumerate(R3):
            lo = r0 * VPW
            hi = min((r1 - 1) * VPW + W + 4, HV - 1)
            nc.vector.tensor_tensor(a27[:, lo:hi], vp7[:, lo:hi],
                                    vp7[:, lo + 1:hi + 1], MIN)
            emit_tail(7, vp7, a27, rows=(r0, r1), b2_pool=0)
            emit_relu(7, 7 + i, rows=(r0, r1))
            if i == 1:
                phead = stats_pool.tile([P, 1], F32, name=f"phead{rep}",
                                        tag="phead")
                nc.vector.tensor_reduce(phead[:], partials[:, 0:9],
                                        mybir.AxisListType.X, ADD)
        ptot = stats_pool.tile([P, 1], F32, name=f"ptot{rep}", tag="ptot")
        nc.vector.tensor_tensor(ptot[:], phead[:], partials[:, 9:10], ADD)

    # ------------- global count -> scale = M / count_ones -------------
    from concourse import bass_isa
    pall = stats_pool.tile([P, 1], F32, name=f"pall{rep}", tag="pall")
    nc.gpsimd.partition_all_reduce(pall[:], ptot[:], channels=P,
                                   reduce_op=bass_isa.ReduceOp.add)
    if single_core:
        tot_sb = pall
    else:
        cc_in = dram_pool.tile([P, 1], F32, name=f"cc_in{rep}", tag="cc_in")
        cc_out = dram_pool.tile([P, 1], F32, name=f"cc_out{rep}",
                                tag="cc_out")
        nc.scalar.dma_start(cc_in[:], pall[:])
        nc.gpsimd.collective_compute(
            "AllReduce", ADD,
            replica_groups=[list(range(N_CORES))],
            ins=[cc_in.opt()], outs=[cc_out.opt()])
        tot_sb = stats_pool.tile([P, 1], F32, name=f"tot{rep}", tag="tot")
        nc.scalar.dma_start(tot_sb[:], cc_out[:])
    # scale = M / count: fold 1/M into the reciprocal input
    rin = stats_pool.tile([P, 1], F32, name=f"rin{rep}", tag="rin")
    nc.vector.tensor_scalar_mul(rin[:], tot_sb[:], 1.0 / COUNT_M)
    scale_sb = stats_pool.tile([P, 1], F32, name=f"scale{rep}", tag="scale")
    nc.vector.reciprocal(scale_sb[:], rin[:])

    # ---------------- phase 2: out = (x*scale)*mask ----------------
    # halves: the first store can launch after half a multiply, and the
    # two HWDGE queues interleave at half-tile granularity
    HX = OPIX // 2
    for t in range(NTILES):
        xt = xts[t]
        eng = nc.vector
        qeng = nc.sync if t % 2 == 0 else nc.scalar
        for h in range(2):
            sl = slice(h * HX, (h + 1) * HX)
            eng.scalar_tensor_tensor(
                xt[:, sl], xt[:, sl], scale_sb[:, 0:1],
                mask_store[:, t * OPIX + h * HX:t * OPIX + (h + 1) * HX],
                MULT, MULT)
            qeng.dma_start(out_ap[bass.ts(t, P), sl], xt[:, sl])


def _get_nc():
    if "nc" not in _CACHE:
        _CACHE["nc"] = _build()
    return _CACHE["nc"]


def kernel(x: np.ndarray, noise: np.ndarray) -> np.ndarray:
    x = np.asarray(x, dtype=np.float32)
    noise = np.asarray(noise, dtype=np.float32)
    assert x.shape == (B, C, H, W) and noise.shape == (B, C, NH, NW)
    nc = _get_nc()
    in_maps = []
    for i in range(N_CORES):
        xs = np.ascontiguousarray(x[i * B_SH:(i + 1) * B_SH]).reshape(
            IMGS, OPIX)
        ns = np.ascontiguousarray(noise[i * B_SH:(i + 1) * B_SH]).reshape(
            IMGS, NPIX)
        in_maps.append({"x": xs, "noise": ns})
    res = run_bass_kernel_spmd(nc, in_maps, list(range(N_CORES)))
    out = np.empty((B, C, H, W), dtype=np.float32)
    for i in range(N_CORES):
        out[i * B_SH:(i + 1) * B_SH] = res.results[i]["out"].reshape(
            B_SH, C, H, W)
    return out


# revision 22
# speedup vs baseline: 1.2070x; 1.1821x over previous
"""DropBlock kernel for Trainium2, 8 NeuronCores, batch-sharded data parallel.

Reference computation (B,C,H,W = 128,64,56,56, block=5, gamma=0.02):
    drop    = (noise < gamma)                       # (B,C,52,52) corner drops
    dilated = maxpool5x5_full_pad(drop)             # (B,C,56,56)
    block_mask = 1 - dilated
    out = block_mask * x * (numel / sum(block_mask))

Kernel formulation (exact -- all intermediates are small integers):
    m = (noise < gamma) in {0,1}                    # Pool tensor_scalar
    C = conv5x5_fullpad(m)   (separable: vertical then horizontal box sum)
    block_mask = (C < 0.5)   == (C == 0) == not dilated
    count = sum(block_mask) via fused ACT accum; AllReduce across 8 cores.

Engine assignment (each phase-1 stage on its own engine, pipelined by
tile; the Pool engine only supports TensorScalar/memset/ISA ops and the
PE only contracts over partitions -- both constraints shaped this):
    Pool: m = tensor_scalar(noise, is_lt gamma)  (the only legal Pool op)
    PE  : vertical 5-row box sum as 5 identity-matmuls with row-shifted
          moving APs accumulated into PSUM (416-col row-aligned chunks;
          contraction over partitions is a no-op via the identity)
    ACT : PSUM -> SBUF bf16 drain into the 60-wide zero-padded layout,
          and the final mask write (fp8) + fused count accumulation
    DVE : horizontal box sum (3 log-step adds), threshold (tensor_scalar
          is_lt at 4x), and the phase-2 (x*scale)*mask multiplies
Tile 7's horizontal chain runs in row-thirds to shorten the critical
path into the count -> AllReduce -> scale; stores start right after.
"""

import sys

sys.path.insert(0, "/opt/trn_rl_repo")

import numpy as np

import concourse.bacc as bacc
import concourse.bass as bass
import concourse.tile as tile
import concourse.mybir as mybir
from concourse.masks import make_identity
from concourse.bass_utils import run_bass_kernel_spmd

N_CORES = 8
B, C, H, W = 128, 64, 56, 56
BLK = 5
GAMMA = 0.02
NH, NW = H - (BLK - 1), W - (BLK - 1)  # 52, 52 noise dims
B_SH = B // N_CORES  # 16 batches per core
IMGS = B_SH * C  # 1024 images per core
P = 128  # partitions
NTILES = IMGS // P  # 8 tiles per core
NPIX = NH * NW  # 2704 noise pixels/image
OPIX = H * W  # 3136 out pixels/image
TROWS = NH + 2 * (BLK - 1)  # 60 rows in zero-padded indicator buffer
TFLAT = TROWS * NW  # 3120
VPW = NW + 2 * (BLK - 1)  # 60 cols in zero-padded horizontal buffer
HV = H * VPW  # 3360
COUNT_M = float(B * C * H * W)  # 25690112.0

F32 = mybir.dt.float32
BF16 = mybir.dt.bfloat16
FP8 = mybir.dt.float8e4
MULT = mybir.AluOpType.mult
ADD = mybir.AluOpType.add
IS_LT = mybir.AluOpType.is_lt
IDENT = mybir.ActivationFunctionType.Identity

CHROWS = 8  # V rows per PE/PSUM chunk (416 cols = 1 PSUM bank)
NCHUNK = H // CHROWS  # 7 chunks per tile
CHW = CHROWS * NW  # 416

_CACHE = {}


def _build(single_core=False, repeat=1, no_cc=False):
    """Build + compile the SPMD bass module once.

    single_core=True builds a collective-free variant (per-core count used
    as the global count) for cost-model simulation only.  repeat>1 unrolls
    the pipeline k times (benchmarking only).  no_cc=True skips the
    AllReduce on the 8-core build (timing probe only).
    """
    nc = bacc.Bacc("TRN2", target_bir_lowering=False, debug=False,
                   num_devices=1 if single_core else N_CORES)
    noise_ap = nc.dram_tensor("noise", [IMGS, NPIX], F32,
                              kind="ExternalInput").ap()
    x_ap = nc.dram_tensor("x", [IMGS, OPIX], F32, kind="ExternalInput").ap()
    out_ap = nc.dram_tensor("out", [IMGS, OPIX], F32,
                            kind="ExternalOutput").ap()

    with tile.TileContext(nc) as tc:
        with (
            tc.tile_pool(name="nraw", bufs=4) as nraw_pool,
            tc.tile_pool(name="work", bufs=1) as work_pool,
            tc.tile_pool(name="vp", bufs=2) as vp_pool,
            tc.tile_pool(name="dmin", bufs=2) as dmin_pool,
            tc.tile_pool(name="mask", bufs=1) as mask_pool,
            tc.tile_pool(name="stats", bufs=1) as stats_pool,
            tc.tile_pool(name="xio", bufs=8) as x_pool,
            tc.tile_pool(name="psum", bufs=8, space="PSUM") as psum_pool,
            tc.tile_pool(name="dram", bufs=1, space="DRAM") as dram_pool,
        ):
            mask_store = mask_pool.tile([P, NTILES * OPIX], FP8)
            # count columns: tiles 0..6 -> 0..6, tile-7 thirds -> 7..9
            partials = stats_pool.tile([P, 10], F32)
            ident = stats_pool.tile([P, P], BF16)
            make_identity(nc, ident[:])
            # warm the ACT Identity table during the DMA lead-in
            warm = stats_pool.tile([P, 1], F32)
            warm2 = stats_pool.tile([P, 1], F32)
            nc.scalar.activation(warm[:], warm[:], IDENT,
                                 accum_out=warm2[:, 0:1])

            pools = dict(nraw=nraw_pool, work=work_pool, vp=vp_pool,
                         dmin=dmin_pool, x=x_pool, psum=psum_pool,
                         stats=stats_pool, dram=dram_pool)
            for rep in range(repeat):
                _emit_once(nc, tc, noise_ap, x_ap, out_ap, mask_store,
                           partials, ident, pools, single_core or no_cc,
                           rep)

    nc.compile()
    return nc


def _emit_once(nc, tc, noise_ap, x_ap, out_ap, mask_store, partials, ident,
               pools, single_core, rep):
    nraw_pool = pools["nraw"]
    work_pool = pools["work"]
    vp_pool = pools["vp"]
    dmin_pool = pools["dmin"]
    x_pool = pools["x"]
    stats_pool = pools["stats"]
    dram_pool = pools["dram"]
    psum_pool = pools["psum"]

    xts = {}
    state = {}
    pre = {}
    # pre-allocate the pad-carrying buffers and memset the ZERO pads once,
    # up-front (in-loop memsets would queue behind real Pool/DVE work)
    pre["mb"] = [work_pool.tile([P, TFLAT], BF16, name=f"mbp{rep}_{i}",
                                tag="mb", bufs=3) for i in range(3)]
    pre["vp"] = [vp_pool.tile([P, HV], BF16, name=f"vpp{rep}_{i}",
                              tag="vp") for i in range(2)]
    if rep == 0:
        for mbp in pre["mb"]:
            nc.gpsimd.memset(mbp[:, 0:(BLK - 1) * NW], 0.0)
            nc.gpsimd.memset(mbp[:, (NH + BLK - 1) * NW:TFLAT], 0.0)
        for vpp in pre["vp"]:
            v3 = vpp[:].rearrange("p (h w) -> p h w", w=VPW)
            nc.gpsimd.memset(v3[:, :, 0:BLK - 1], 0.0)
            nc.gpsimd.memset(v3[:, :, W:VPW], 0.0)

    def emit_front(t):
        """noise loads, x prefetch, Pool indicator, PE conv, ACT drain."""
        nh0 = nraw_pool.tile([P, NPIX // 2], F32, name=f"nh{rep}_{t}a",
                             tag="nh")
        nh1 = nraw_pool.tile([P, NPIX // 2], F32, name=f"nh{rep}_{t}b",
                             tag="nh")
        nc.sync.dma_start(nh0[:], noise_ap[bass.ts(t, P), 0:NPIX // 2])
        nc.sync.dma_start(nh1[:], noise_ap[bass.ts(t, P), NPIX // 2:NPIX])
        # x loads ride the other HWDGE queue (ACT's), in halves
        xts[t] = x_pool.tile([P, OPIX], F32, name=f"xt{rep}_{t}", tag="xt")
        with tc.tile_wait_until(t * 10.0 + 5.0 if t < NTILES - 1 else 85.0):
            nc.scalar.dma_start(xts[t][:, 0:OPIX // 2],
                                x_ap[bass.ts(t, P), 0:OPIX // 2])
            nc.scalar.dma_start(xts[t][:, OPIX // 2:OPIX],
                                x_ap[bass.ts(t, P), OPIX // 2:OPIX])

        # drop indicator m in {0,1}, rows 4..55 of the 60-row zero-padded
        # buffer; exact f32 compare against gamma, bf16 out (Pool engine)
        mb = pre["mb"][t] if t < 3 else work_pool.tile(
            [P, TFLAT], BF16, name=f"mb{rep}_{t}", tag="mb", bufs=3)
        nc.gpsimd.tensor_scalar(mb[:, (BLK - 1) * NW:30 * NW], nh0[:],
                                GAMMA, None, IS_LT)
        nc.gpsimd.tensor_scalar(mb[:, 30 * NW:(NH + BLK - 1) * NW], nh1[:],
                                GAMMA, None, IS_LT)

        # vertical 5-row box sum on PE: V[i] = sum_j m[i + 52j], computed
        # as 5 identity-matmuls with row-shifted moving APs accumulated in
        # PSUM, in 8-row chunks (416 cols = 1 bank); ACT drains each chunk
        # into the padded horizontal buffer as bf16 (values 0..5, exact)
        vp = pre["vp"][t] if t < 2 else vp_pool.tile(
            [P, HV], BF16, name=f"vp{rep}_{t}", tag="vp")
        vp3 = vp[:].rearrange("p (h w) -> p h w", w=VPW)
        for c in range(NCHUNK):
            pt = psum_pool.tile([P, CHW], F32, name=f"ps{rep}_{t}_{c}",
                                tag="ps")
            for j in range(BLK):
                nc.tensor.matmul(
                    pt[:], ident[:],
                    mb[:, c * CHW + NW * j:c * CHW + NW * j + CHW],
                    start=(j == 0), stop=(j == BLK - 1))
            pt3 = pt[:].rearrange("p (h w) -> p h w", w=NW)
            dst = vp3[:, c * CHROWS:(c + 1) * CHROWS, BLK - 1:BLK - 1 + NW]
            nc.scalar.activation(dst, pt3[:, :, :], IDENT)
        state[t] = {"vp": vp}
        return vp

    def emit_h(t, rows=None):
        """horizontal box sum + threshold on DVE for row range [r0, r1)."""
        r0, r1 = (0, H) if rows is None else rows
        vp = state[t]["vp"]
        a2 = state[t].get("a2")
        if a2 is None:
            a2 = work_pool.tile([P, HV], BF16, name=f"a2{rep}_{t}",
                                tag="a2", bufs=2)
            state[t]["a2"] = a2
            b2 = work_pool.tile([P, HV], BF16, name=f"b2{rep}_{t}",
                                tag="b2", bufs=1)
            state[t]["b2"] = b2
            cs = dmin_pool.tile([P, OPIX], BF16, name=f"cs{rep}_{t}",
                                tag="cs")
            state[t]["cs"] = cs
        b2 = state[t]["b2"]
        cs = state[t]["cs"]
        alo, ahi = r0 * VPW, min((r1 - 1) * VPW + W + 2, HV - 1)
        nc.vector.tensor_tensor(a2[:, alo:ahi], vp[:, alo:ahi],
                                vp[:, alo + 1:ahi + 1], ADD)
        blo, bhi = r0 * VPW, min((r1 - 1) * VPW + W, HV - 2)
        nc.vector.tensor_tensor(b2[:, blo:bhi], a2[:, blo:bhi],
                                a2[:, blo + 2:bhi + 2], ADD)
        b23 = b2[:].rearrange("p (h w) -> p h w", w=VPW)
        vp3 = vp[:].rearrange("p (h w) -> p h w", w=VPW)
        cs3 = cs[:].rearrange("p (h w) -> p h w", w=W)
        nc.vector.tensor_tensor(cs3[:, r0:r1, :], b23[:, r0:r1, 0:W],
                                vp3[:, r0:r1, BLK - 1:VPW], ADD)
        # block_mask = (C == 0): threshold in-place at 4x
        nc.vector.tensor_scalar(cs3[:, r0:r1, :], cs3[:, r0:r1, :], 0.5,
                                None, IS_LT)
        return cs

    def emit_mask(t, col, rows=None):
        """mask -> fp8 store + fused count accum on ACT."""
        r0, r1 = (0, H) if rows is None else rows
        cs = state[t]["cs"]
        msl = mask_store[:].rearrange("p (t h w) -> p t h w", t=NTILES, w=W)
        cs3 = cs[:].rearrange("p (h w) -> p h w", w=W)
        nc.scalar.activation(msl[:, t, r0:r1, :], cs3[:, r0:r1, :], IDENT,
                             accum_out=partials[:, col:col + 1])

    RB = 10.0  # pseudo-ms per tile: scheduler ordering only
    for t in range(NTILES):
        with tc.tile_wait_until(t * RB):
            emit_front(t)
        if t >= 1 and t - 1 < NTILES - 1:
            with tc.tile_wait_until(t * RB + 2.5):
                emit_h(t - 1)
            with tc.tile_wait_until(t * RB + 3.0):
                emit_mask(t - 1, t - 1)

    # tile 7 in row-thirds to shorten the critical path into the count
    R3 = [(0, 19), (19, 38), (38, H)]
    with tc.tile_wait_until(8 * RB):
        phead = None
        for i, (r0, r1) in enumerate(R3):
            emit_h(7, rows=(r0, r1))
            emit_mask(7, 7 + i, rows=(r0, r1))
            if i == 1:
                phead = stats_pool.tile([P, 1], F32, name=f"phead{rep}",
                                        tag="phead")
                nc.vector.tensor_reduce(phead[:], partials[:, 0:9],
                                        mybir.AxisListType.X, ADD)
        ptot = stats_pool.tile([P, 1], F32, name=f"ptot{rep}", tag="ptot")
        nc.vector.tensor_tensor(ptot[:], phead[:], partials[:, 9:10], ADD)

        # ---------- global count -> scale = M / count_ones ----------
        from concourse import bass_isa
        pall = stats_pool.tile([P, 1], F32, name=f"pall{rep}", tag="pall")
        nc.gpsimd.partition_all_reduce(pall[:], ptot[:], channels=P,
                                       reduce_op=bass_isa.ReduceOp.add)
        if single_core:
            tot_sb = pall
        else:
            cc_in = dram_pool.tile([P, 1], F32, name=f"cc_in{rep}",
                                   tag="cc_in")
            cc_out = dram_pool.tile([P, 1], F32, name=f"cc_out{rep}",
                                    tag="cc_out")
            nc.scalar.dma_start(cc_in[:], pall[:])
            nc.gpsimd.collective_compute(
                "AllReduce", ADD,
                replica_groups=[list(range(N_CORES))],
                ins=[cc_in.opt()], outs=[cc_out.opt()])
            tot_sb = stats_pool.tile([P, 1], F32, name=f"tot{rep}",
                                     tag="tot")
            nc.scalar.dma_start(tot_sb[:], cc_out[:])
        # scale = M / count: fold 1/M into the reciprocal input
        rin = stats_pool.tile([P, 1], F32, name=f"rin{rep}", tag="rin")
        nc.vector.tensor_scalar_mul(rin[:], tot_sb[:], 1.0 / COUNT_M)
        scale_sb = stats_pool.tile([P, 1], F32, name=f"scale{rep}",
                                   tag="scale")
        nc.vector.reciprocal(scale_sb[:], rin[:])

    # ---------------- phase 2: out = (x*scale)*mask ----------------
    # halves: the first store launches after half a multiply; stores
    # alternate the two HWDGE queues
    HX = OPIX // 2
    with tc.tile_wait_until(9 * RB):
        for t in range(NTILES):
            xt = xts[t]
            qeng = nc.sync if t % 2 == 0 else nc.scalar
            for h in range(2):
                sl = slice(h * HX, (h + 1) * HX)
                nc.vector.scalar_tensor_tensor(
                    xt[:, sl], xt[:, sl], scale_sb[:, 0:1],
                    mask_store[:,
                               t * OPIX + h * HX:t * OPIX + (h + 1) * HX],
                    MULT, MULT)
                qeng.dma_start(out_ap[bass.ts(t, P), sl], xt[:, sl])


def _get_nc():
    if "nc" not in _CACHE:
        _CACHE["nc"] = _build()
    return _CACHE["nc"]


def kernel(x: np.ndarray, noise: np.ndarray) -> np.ndarray:
    x = np.asarray(x, dtype=np.float32)
    noise = np.asarray(noise, dtype=np.float32)
    assert x.shape == (B, C, H, W) and noise.shape == (B, C, NH, NW)
    nc = _get_nc()
    in_maps = []
    for i in range(N_CORES):
        xs = np.ascontiguousarray(x[i * B_SH:(i + 1) * B_SH]).reshape(
            IMGS, OPIX)
        ns = np.ascontiguousarray(noise[i * B_SH:(i + 1) * B_SH]).reshape(
            IMGS, NPIX)
        in_maps.append({"x": xs, "noise": ns})
    res = run_bass_kernel_spmd(nc, in_maps, list(range(N_CORES)))
    out = np.empty((B, C, H, W), dtype=np.float32)
    for i in range(N_CORES):
        out[i * B_SH:(i + 1) * B_SH] = res.results[i]["out"].reshape(
            B_SH, C, H, W)
    return out
